# revision 41
# baseline (speedup 1.0000x reference)
"""Multi-head attention (b=2, c=768, s=2048, 8 heads, d=96) on 8 TRN2 NeuronCores.

Sharding: batch x head-group tensor parallel. Core i handles batch i//4 and
heads {2*(i%4), 2*(i%4)+1}. Each core computes its two heads' attention plus
their contribution to the output projection; the host sums the 4 partial
outputs per batch element (the all-reduce of the sharding hint, done host-side
since the kernel returns full outputs anyway).

Per-core pipeline (all matmuls float32r ~ tf32 precision, fp32 accumulate;
measured end-to-end rel err ~6e-4):
  qT/kT = W^T @ xT          (96, 2048): x arrives already transposed as (c, s)
  S^T[j,i] = k_j . q_i      scores computed TRANSPOSED (j on partitions) so the
                            P @ V contraction needs no on-chip transposes
  P = exp(S^T)              no max-subtraction: |S| <= ~15 << fp32 overflow
  O~ = [V;1]^T @ P          ones column appended to V yields the softmax
                            denominator as PSUM row 96 of the same matmul
  o = O~[0:96] * (1/den)    denominator broadcast across partitions via a
                            K=1 ones matmul on the PE
  out += W_out_h^T @ o      accumulated over the core's 2 heads in PSUM

Schedule (TimelineSim ~133us/core; PE busy ~98us = N-column roofline of the
~229K streamed PE cycles):
  - x is DMA'd in (c-tile, 512-query-slice) chunks on the HWDGE queue while
    weights load as single batched DMAs on the SWDGE queue, so the first
    projection matmul starts ~2us in;
  - all ps_proj-pool users are emitted in exact x-arrival order (a DMA-blocked
    projection holding a pool slot would otherwise head-of-line-block ready
    work);
  - exp runs per (128, 1024) PSUM group (2 banks, 2 bufs) feeding the PV
    accumulation; qT projections for later slices are emitted after each
    attention block so the PE fills exp-wait gaps with projection work.
"""

import numpy as np

N_CORES = 8
B, C, S = 2, 768, 2048
H, D = 8, 96
CT = C // 128          # 6 c-tiles
IT = S // 512          # 4 query slices
JT = S // 128          # 16 key tiles
JG = JT // 2           # 8 exp groups of 2 key tiles

_RUNNER = None


def _split_sync_waits(nc, mybir, max_waits=1):
    """This walrus build rejects instructions carrying more than one sem wait
    (setupSyncWait: 'Too many sync wait commands'). Split excess waits onto
    same-engine NoOps inserted immediately before the instruction."""
    for bb in nc.main_func.blocks:
        insts = bb.instructions
        i = 0
        while i < len(insts):
            inst = insts[i]
            si = inst.sync_info
            if si is not None and si.on_wait and len(si.on_wait) > max_waits:
                waits = list(si.on_wait)
                keep = waits[-max_waits:]
                extra = waits[:-max_waits]
                pos = i
                while extra:
                    chunk, extra = extra[:max_waits], extra[max_waits:]
                    nop = mybir.InstNoOp(
                        name=nc.get_next_instruction_name(),
                        sync_info=mybir.SyncInfo(on_wait=chunk, on_update=[]),
                        engine=inst.engine,
                        bass_nofuse=True,
                    )
                    insts.insert(pos, nop)
                    pos += 1
                    i += 1
                si.on_wait = keep
            i += 1


DEFAULT_CFG = dict(
    dma_order="B",        # "A": wk, x(all), wq, wv, wo ; "B": wk, x0, wq, wv, x1-3, wo
    b_phase="stream_part",  # projections emitted in x-arrival order, qT 1+ as fillers
    ps_proj=2, ps_attn=2, ps_o=2,
    attn_scheme="2x2",    # "2x2": 8 groups of 2 from one pool; "3x1": 3 pools bufs=1;
                          # "mix": 7 groups of 2 (pool A bufs=2) + 2 of 1 (pool B bufs=1)
    tail_split=False, tail_pin=False,
    loop_n=1,             # benchmark mode: repeat the whole body in a HW loop
)


def _build_nc(cfg=None):
    import concourse.bass as bass
    import concourse.tile as tile
    import concourse.mybir as mybir
    from concourse.tile import add_dep_helper

    cfg = {**DEFAULT_CFG, **(cfg or {})}

    f32 = mybir.dt.float32
    f32r = mybir.dt.float32r
    EXP = mybir.ActivationFunctionType.Exp

    nc = bass.Bass(num_devices=N_CORES)
    x = nc.declare_dram_parameter("x", [C, S], f32, isOutput=False)
    wq = nc.declare_dram_parameter("wq", [C, 2 * D], f32, isOutput=False)
    wk = nc.declare_dram_parameter("wk", [C, 2 * D], f32, isOutput=False)
    wv = nc.declare_dram_parameter("wv", [C, 2 * D], f32, isOutput=False)
    wo = nc.declare_dram_parameter("wo", [2 * D, C], f32, isOutput=False)
    out = nc.declare_dram_parameter("out", [C, S], f32, isOutput=True)

    with tile.TileContext(nc) as tc:
        with (
            tc.tile_pool(name="sb_x", bufs=1) as sb_x,
            tc.tile_pool(name="sb_w", bufs=1) as sb_w,
            tc.tile_pool(name="sb_qk", bufs=1) as sb_qk,
            tc.tile_pool(name="sb_v", bufs=1) as sb_v,
            tc.tile_pool(name="sb_p", bufs=4) as sb_p,
            tc.tile_pool(name="sb_o", bufs=3) as sb_o,
            tc.tile_pool(name="sb_m", bufs=2) as sb_m,
            tc.tile_pool(name="sb_oc", bufs=3) as sb_oc,
            tc.tile_pool(name="sb_oc0", bufs=6) as sb_oc0,
            tc.tile_pool(name="ps_proj", bufs=cfg["ps_proj"], space="PSUM") as ps_proj,
            tc.tile_pool(name="ps_attn", bufs=cfg["ps_attn"], space="PSUM") as ps_attn,
            tc.tile_pool(name="ps_attn2", bufs=1, space="PSUM") as ps_attn2,
            tc.tile_pool(name="ps_attn3", bufs=1, space="PSUM") as ps_attn3,
            tc.tile_pool(name="ps_o", bufs=cfg["ps_o"], space="PSUM") as ps_o,
        ):
          import contextlib
          loop_ctx = tc.For_i(0, cfg["loop_n"], 1) if cfg["loop_n"] > 1 else contextlib.nullcontext()
          with loop_ctx:
            # fp32 constants (memset can't target f32r); DVE copies round to f32r
            czero = sb_w.tile([128, 64], f32, name="czero")
            nc.vector.memset(czero[:], 0.0)
            cone = sb_w.tile([128, 96], f32, name="cone")
            nc.vector.memset(cone[:], 1.0)
            ones1 = sb_w.tile([1, D], f32r, name="ones1")
            nc.vector.tensor_copy(ones1[:], cone[0:1, :])

            # ---- loads ----
            def load_x_slice(xt, isl, eng=None):
                eng = eng or nc.sync
                for ct in range(CT):
                    eng.dma_start(
                        xt_c[(ct, isl)][:],
                        x[ct * 128:(ct + 1) * 128, isl * 512:(isl + 1) * 512].bitcast(f32r),
                    )

            # weights load as one DMA each on the SWDGE queue (gpsimd), in
            # parallel with x streaming on the HWDGE queue (sync)
            def load_wk():
                tk = sb_w.tile([128, CT, 2 * D], f32r, name="wk")
                nc.gpsimd.dma_start(
                    tk[:], wk.rearrange("(ct p) c -> p ct c", p=128).bitcast(f32r)
                )
                return [tk[:, ct, :] for ct in range(CT)]

            def load_wq_wv():
                tq = sb_w.tile([128, CT, 2 * D], f32r, name="wq")
                nc.gpsimd.dma_start(
                    tq[:], wq.rearrange("(ct p) c -> p ct c", p=128).bitcast(f32r)
                )
                tv = sb_w.tile([128, CT, 256], f32r, name="wv")
                nc.vector.tensor_copy(
                    tv[:, :, 2 * D:256],
                    czero[:, None, :].broadcast_to([128, CT, 64]),
                )
                nc.gpsimd.dma_start(
                    tv[:, :, 0:2 * D],
                    wv.rearrange("(ct p) c -> p ct c", p=128).bitcast(f32r),
                )
                return ([tq[:, ct, :] for ct in range(CT)],
                        [tv[:, ct, :] for ct in range(CT)])

            def load_wo():
                t = sb_w.tile([D, 2, C], f32r, name="wo")
                nc.gpsimd.dma_start(
                    t[:], wo.rearrange("(h p) c -> p h c", p=D).bitcast(f32r)
                )
                return [t[:, h, :] for h in range(2)]

            xt_c = {(ct, w): sb_x.tile([128, 512], f32r, name=f"xt{ct}_{w}")
                    for ct in range(CT) for w in range(IT)}

            class _XtView:
                """xt[ct][:, a:b] view over per-(ct, slice) tiles; slices must
                stay within one 512-wide chunk."""
                def __init__(self, ct):
                    self.ct = ct
                def __getitem__(self, key):
                    rows, cols = key
                    a, b = cols.start or 0, cols.stop
                    w, off = divmod(a, 512)
                    assert b - a <= 512 and off + (b - a) <= 512
                    return xt_c[(self.ct, w)][rows, off:off + (b - a)]

            xt = [_XtView(ct) for ct in range(CT)]
            if cfg["dma_order"] == "A":
                wk_t = load_wk()
                for isl in range(IT):
                    load_x_slice(xt, isl)
                wq_t, wv_t = load_wq_wv()
                wo_t = load_wo()
            else:
                wk_t = load_wk()
                load_x_slice(xt, 0)
                wq_t, wv_t = load_wq_wv()
                for isl in range(1, IT):
                    load_x_slice(xt, isl)
                wo_t = load_wo()

            qT = [sb_qk.tile([D, S], f32r, name=f"qT{h}") for h in range(2)]
            kT = [sb_qk.tile([D, S], f32r, name=f"kT{h}") for h in range(2)]
            v_cat = [sb_v.tile([128, JT, D + 1], f32r, name=f"v{h}") for h in range(2)]

            def proj_qk(h, isl, w_t, dst, pin_after=None):
                acc = ps_proj.tile([128, 512], f32, name="ps_proj")
                for ct in range(CT):
                    mm = nc.tensor.matmul(
                        acc[0:D, :],
                        w_t[ct][:, h * D:(h + 1) * D],
                        xt[ct][:, isl * 512:(isl + 1) * 512],
                        start=(ct == 0), stop=(ct == CT - 1),
                    )
                    if ct == 0 and pin_after is not None:
                        add_dep_helper(mm.ins, pin_after.ins, sync=True,
                                       reason="pin filler projection into block")
                nc.vector.tensor_copy(dst[:, isl * 512:(isl + 1) * 512], acc[0:D, :])

            def proj_v(jt):
                accv = ps_proj.tile([128, 512], f32, name="ps_proj")
                for ct in range(CT):
                    nc.tensor.matmul(
                        accv[:, 0:256],
                        xt[ct][:, jt * 128:(jt + 1) * 128],
                        wv_t[ct][:],
                        start=(ct == 0), stop=(ct == CT - 1),
                    )
                for h in range(2):
                    nc.vector.tensor_copy(v_cat[h][:, jt, 0:D], accv[:, h * D:(h + 1) * D])
                    nc.vector.tensor_copy(v_cat[h][:, jt, D:D + 1], cone[:, jt:jt + 1])

            if cfg["b_phase"] == "stream":
                for w in range(IT):
                    proj_qk(0, w, wk_t, kT[0])
                    for jt in range(4 * w, 4 * w + 4):
                        proj_v(jt)
                    proj_qk(1, w, wk_t, kT[1])
                    proj_qk(0, w, wq_t, qT[0])
                    proj_qk(1, w, wq_t, qT[1])
            elif cfg["b_phase"] == "stream_part":
                # emit ps_proj users in exact x-slice arrival order so a
                # DMA-blocked projection never holds a slot that a ready one
                # needs (head-of-line blocking); qT slices 1+ stay as in-block
                # fillers
                for w in range(IT):
                    proj_qk(0, w, wk_t, kT[0])
                    if w == 0:
                        proj_qk(0, 0, wq_t, qT[0])
                    for jt in range(4 * w, 4 * w + 4):
                        proj_v(jt)
                    proj_qk(1, w, wk_t, kT[1])
                    if w == 0:
                        proj_qk(1, 0, wq_t, qT[1])
            else:
                for isl in range(IT):
                    proj_qk(0, isl, wk_t, kT[0])
                proj_qk(0, 0, wq_t, qT[0])

            # ---- attention + output projection ----
            scheme = cfg["attn_scheme"]
            if scheme == "2x2":
                GROUPS = [list(range(g * 2, g * 2 + 2)) for g in range(JG)]
            elif scheme == "mix":
                GROUPS = [list(range(g * 2, g * 2 + 2)) for g in range(7)] + [[14], [15]]
            else:  # 3x1
                GROUPS = [list(range(g * 2, g * 2 + 2)) for g in range(JG)]

            def alloc_sg(gi, width):
                if scheme == "3x1":
                    pool = (ps_attn, ps_attn2, ps_attn3)[gi % 3]
                    return pool.tile([128, width], f32, name=f"sg{gi % 3}")
                if scheme == "mix" and width == 512:
                    return ps_attn2.tile([128, 512], f32, name="sg_small")
                return ps_attn.tile([128, 1024], f32, name="ps_attn")

            def attention_block(h, isl):
                Oacc = ps_o.tile([D + 1, 512], f32, name="ps_o")
                exp0 = None
                for gi, jts in enumerate(GROUPS):
                    if cfg["b_phase"] == "filler" and isl == 0 and h == 0:
                        for jt in jts:
                            proj_v(jt)
                    width = 512 * len(jts)
                    sg = alloc_sg(gi, width)
                    for t, jt in enumerate(jts):
                        nc.tensor.matmul(
                            sg[:, t * 512:(t + 1) * 512],
                            kT[h][:, jt * 128:(jt + 1) * 128],
                            qT[h][:, isl * 512:(isl + 1) * 512],
                            start=True, stop=True,
                        )
                    pt = sb_p.tile([128, 1024], f32r, name="pt")
                    e = nc.scalar.activation(pt[:, 0:width], sg[:, 0:width], EXP)
                    if exp0 is None:
                        exp0 = e
                    for t, jt in enumerate(jts):
                        nc.tensor.matmul(
                            Oacc[:],
                            v_cat[h][:, jt, :],
                            pt[:, t * 512:(t + 1) * 512],
                            start=(jt == 0), stop=(jt == JT - 1),
                        )
                return Oacc, exp0

            def normalize(Oacc):
                recip = sb_m.tile([1, 512], f32, name="recip")
                nc.vector.reciprocal(recip[:], Oacc[D:D + 1, :])
                recip_r = sb_m.tile([1, 512], f32r, name="recip_r")
                nc.vector.tensor_copy(recip_r[:], recip[:])
                bc_ps = ps_proj.tile([128, 512], f32, name="ps_proj")
                nc.tensor.matmul(bc_ps[0:D, :], ones1[:], recip_r[:], start=True, stop=True)
                bc = sb_m.tile([D, 512], f32, name="bc")
                nc.vector.tensor_copy(bc[:], bc_ps[0:D, :])
                o = sb_o.tile([D, 512], f32r, name="o_n")
                nc.vector.tensor_mul(o[:], Oacc[0:D, :], bc[:])
                return o

            filler = cfg["b_phase"] == "filler"
            part = cfg["b_phase"] == "stream_part"
            for isl in range(IT):
                last = isl == IT - 1
                O0, e0 = attention_block(0, isl)
                if filler:
                    if isl == 0:
                        for isl2 in range(IT):
                            proj_qk(1, isl2, wk_t, kT[1])
                        proj_qk(1, 0, wq_t, qT[1])
                    else:
                        proj_qk(1, isl, wq_t, qT[1])
                elif part and isl > 0:
                    proj_qk(1, isl, wq_t, qT[1])
                o0 = normalize(O0)

                oc0 = None
                if last and cfg["tail_split"]:
                    oc0 = []

                O1, e1 = attention_block(1, isl)
                if (filler or part) and not last:
                    proj_qk(0, isl + 1, wq_t, qT[0])
                if oc0 is not None:
                    for ct in range(CT):
                        po = ps_proj.tile([128, 512], f32, name="ps_proj")
                        mm = nc.tensor.matmul(
                            po[:], wo_t[0][:, ct * 128:(ct + 1) * 128], o0[:],
                            start=True, stop=True,
                        )
                        if ct == 0 and cfg["tail_pin"]:
                            add_dep_helper(mm.ins, e1.ins, sync=True,
                                           reason="pin tail h0 out-proj into h1 block")
                        t0 = sb_oc0.tile([128, 512], f32, name="oc0")
                        nc.vector.tensor_copy(t0[:], po[:])
                        oc0.append(t0)
                o1 = normalize(O1)

                for ct in range(CT):
                    if last and ct % 2 == 1:
                        po = ps_attn.tile([128, 1024], f32, name="ps_attn")[:, 0:512]
                    else:
                        po = ps_proj.tile([128, 512], f32, name="ps_proj")
                    if oc0 is not None:
                        nc.tensor.matmul(
                            po[:], wo_t[1][:, ct * 128:(ct + 1) * 128], o1[:],
                            start=True, stop=True,
                        )
                        oc = sb_oc.tile([128, 512], f32, name="oc")
                        nc.vector.tensor_add(oc[:], po[:], oc0[ct][:])
                    else:
                        for h, o in ((0, o0), (1, o1)):
                            nc.tensor.matmul(
                                po[:],
                                wo_t[h][:, ct * 128:(ct + 1) * 128],
                                o[:],
                                start=(h == 0), stop=(h == 1),
                            )
                        oc = sb_oc.tile([128, 512], f32, name="oc")
                        nc.vector.tensor_copy(oc[:], po[:])
                    nc.sync.dma_start(
                        out[ct * 128:(ct + 1) * 128, isl * 512:(isl + 1) * 512], oc[:]
                    )

    _split_sync_waits(nc, mybir)
    return nc


class _Runner:
    """Compile once, run many. Mirrors run_bass_via_pjrt's multi-core path but
    keeps the jitted executable cached across calls."""

    def __init__(self, cfg=None):
        import jax
        import concourse.mybir as mybir
        from concourse import bass2jax
        from jax.sharding import Mesh, PartitionSpec
        from jax.experimental.shard_map import shard_map

        self.jax = jax
        nc = _build_nc(cfg)
        self.nc = nc
        bass2jax.install_neuronx_cc_hook()

        in_names, out_names, out_avals = [], [], []
        for alloc in nc.m.functions[0].allocations:
            if not isinstance(alloc, mybir.MemoryLocationSet):
                continue
            name = alloc.memorylocations[0].name
            if alloc.kind == "ExternalInput":
                if nc.partition_id_tensor is None or name != nc.partition_id_tensor.name:
                    in_names.append(name)
            elif alloc.kind == "ExternalOutput":
                out_names.append(name)
                out_avals.append(
                    jax.core.ShapedArray(tuple(alloc.tensor_shape), mybir.dt.np(alloc.dtype))
                )
        self.in_names = in_names
        self.out_names = out_names
        partition_name = nc.partition_id_tensor.name if nc.partition_id_tensor else None
        all_names = tuple(in_names + out_names + ([partition_name] if partition_name else []))

        def _body(*args):
            operands = list(args)
            if partition_name is not None:
                operands.append(bass2jax.partition_id_tensor())
            outs = bass2jax._bass_exec_p.bind(
                *operands,
                out_avals=tuple(out_avals),
                in_names=all_names,
                out_names=tuple(out_names),
                lowering_input_output_aliases=(),
                sim_require_finite=True,
                sim_require_nnan=True,
                nc=nc,
            )
            return tuple(outs)

        devices = jax.devices()[:N_CORES]
        mesh = Mesh(np.asarray(devices), ("core",))
        n_all = len(in_names) + len(out_names)
        self.sharded = jax.jit(
            shard_map(
                _body,
                mesh=mesh,
                in_specs=(PartitionSpec("core"),) * n_all,
                out_specs=(PartitionSpec("core"),) * len(out_names),
                check_rep=False,
            ),
            keep_unused=True,
        )
        self.out_shapes = [tuple(a.shape) for a in out_avals]
        self.out_dtypes = [a.dtype for a in out_avals]

    def run(self, in_maps):
        concat_in = [
            np.concatenate([np.asarray(in_maps[c][n]) for c in range(N_CORES)], axis=0)
            for n in self.in_names
        ]
        concat_zero = [
            np.zeros((N_CORES * s[0], *s[1:]), d)
            for s, d in zip(self.out_shapes, self.out_dtypes)
        ]
        outs = self.sharded(*concat_in, *concat_zero)
        self.jax.block_until_ready(outs)
        return [
            {
                n: np.asarray(outs[i]).reshape(N_CORES, *self.out_shapes[i])[c]
                for i, n in enumerate(self.out_names)
            }
            for c in range(N_CORES)
        ]


def _get_runner():
    global _RUNNER
    if _RUNNER is None:
        _RUNNER = _Runner()
    return _RUNNER


def _shard_inputs(inputs, W_qkv, W_out):
    in_maps = []
    for core in range(N_CORES):
        b, g = divmod(core, 4)
        cols = slice(g * 2 * D, (g + 1) * 2 * D)
        in_maps.append({
            "x": np.ascontiguousarray(inputs[b]),
            "wq": np.ascontiguousarray(W_qkv[:, cols]),
            "wk": np.ascontiguousarray(W_qkv[:, 768:][:, cols]),
            "wv": np.ascontiguousarray(W_qkv[:, 1536:][:, cols]),
            "wo": np.ascontiguousarray(W_out[cols, :]),
        })
    return in_maps


def kernel(inputs, W_qkv, W_out):
    inputs = np.asarray(inputs, dtype=np.float32)
    W_qkv = np.asarray(W_qkv, dtype=np.float32)
    W_out = np.asarray(W_out, dtype=np.float32)
    runner = _get_runner()
    results = runner.run(_shard_inputs(inputs, W_qkv, W_out))
    out = np.zeros((B, C, S), np.float32)
    for core in range(N_CORES):
        out[core // 4] += results[core]["out"]
    return out
